# revision 6
# baseline (speedup 1.0000x reference)
"""Trainium2 Bass kernel: BoundaryActivation, v11.

Per sample: x1 = cummax(x, H), x2 = reverse-cummax(x, H), x3 = cummax(x, W),
x4 = reverse-cummax(x, W); out = conv1x1(concat([x, x1, x2, x3, x4])) + bias.
Data-parallel over batch: B=8 -> 8 NeuronCores.

v11 vs v10: DVE-scan-bound design (scan rate has no fast path, so every
other engine is pushed below the ~140us scan wall):
- Host sends x as bf16 in BOTH layouts (row-major for the W-direction
  scans, T-order for the H-direction scans) -> no on-device transpose
  gather (was 27us ACT) and no f32 input traffic (DMA 163->~85us).
- x-term computed with one/two fp8e4 DoubleRow matmuls (contraction 256
  in one pass at 0.5 cyc/row) from a resident e4m3 copy of x -> PE
  150->~130us. All stationaries are scaled x256 so fp8 weights stay in
  the normal range; drains descale (scale=1/256) and the merge identity
  is 256*I so psum scales stay consistent.
- Output written as bf16 T-order; host converts to f32 row-major.
- Scans in [128, 2048] instructions (16 rows / 16 w-cols per chunk),
  both phases; scan outputs bf16 (exact cummax of bf16 inputs).

Phase 1 (stream, row-major): per 16-row chunk: DMA stage, DVE x3/x4
scans, PE x-DR + x3 + x4 accumulated per 4-row psum group, ACT drain
(scale + bias) -> osb bf16.
Phase 2 (T-order): per 16-w band: DMA T stage, DVE x1/x2 scans, PE
x1 + x2 + merge (256*I @ osb strided view) per 4-w psum group, ACT
drain (scale) -> outstage bf16 -> DMA out.
"""

import numpy as np
import ml_dtypes
from contextlib import ExitStack

import concourse.bass as bass
import concourse.bacc as bacc
import concourse.mybir as mybir
import concourse.tile as tile
from concourse.bass_utils import run_bass_kernel_spmd

F32 = mybir.dt.float32
BF16 = mybir.dt.bfloat16
E4 = mybir.dt.float8e4
AL = mybir.AluOpType
AFT = mybir.ActivationFunctionType

NEG = -3.0e38
WS = 256.0            # global stationary scale (psum = 256 * out)

B = 8
C = 256
H = 128
W = 128
O = 256
HW = H * W            # 16384
NCC = 2               # input channel chunks of 128
NQ = 2                # output channel chunks of 128
CH = 16               # rows (phase 1) / w-cols (phase 2) per chunk
NCHUNK = H // CH      # 8
CPX = CH * W          # 2048 px per chunk
SUB = 4               # psum accumulation-group: 4 rows / 4 w-cols (512 px)
NSUB = CH // SUB      # 4
NXPL = 2              # x-term stationary planes (hi, hi+lo)


def build_program(nxpl=NXPL):
    nc = bacc.Bacc()
    xst_d = nc.declare_dram_parameter("xst", [NCC, 128, HW], BF16, isOutput=False)
    xtt_d = nc.declare_dram_parameter("xtt", [NCC, 128, HW], BF16, isOutput=False)
    xa_d = nc.declare_dram_parameter("xa", [128, NCC, HW], E4, isOutput=False)
    w34_d = nc.declare_dram_parameter("w34", [128, 2 * NCC * NQ * 128], BF16, isOutput=False)
    w12_d = nc.declare_dram_parameter("w12", [128, 2 * NCC * NQ * 128], BF16, isOutput=False)
    wx_d = nc.declare_dram_parameter("wx", [128, nxpl * NQ * NCC * 128], E4, isOutput=False)
    eye_d = nc.declare_dram_parameter("eye", [128, 128], BF16, isOutput=False)
    b_d = nc.declare_dram_parameter("bias", [128, NQ], F32, isOutput=False)
    out_d = nc.declare_dram_parameter("out", [NQ, 128, HW], BF16, isOutput=True)

    with ExitStack() as ctx:
        tc = ctx.enter_context(tile.TileContext(nc))

        const = ctx.enter_context(tc.tile_pool(name="const", bufs=1))
        persist = ctx.enter_context(tc.tile_pool(name="persist", bufs=1))
        stage_p = ctx.enter_context(tc.tile_pool(name="stage", bufs=4))
        scan_p = ctx.enter_context(tc.tile_pool(name="scan", bufs=6))
        outs_p = ctx.enter_context(tc.tile_pool(name="outs", bufs=2))
        psum_p = ctx.enter_context(tc.tile_pool(name="psum", bufs=2, space="PSUM"))

        # ---- constants / residents ----
        # const loads ride the scalar queue so the sync queue carries only
        # stage loads (DVE's critical dependency); wx first (first matmul)
        wx = const.tile([128, nxpl * NQ, NCC, 128], E4, tag="wx")
        nc.scalar.dma_start(wx[:].rearrange("p k c o -> p (k c o)"), wx_d[:])
        w34 = const.tile([128, 2 * NCC * NQ, 128], BF16, tag="w34")
        nc.scalar.dma_start(w34[:].rearrange("p k o -> p (k o)"), w34_d[:])
        w12 = const.tile([128, 2 * NCC * NQ, 128], BF16, tag="w12")
        nc.scalar.dma_start(w12[:].rearrange("p k o -> p (k o)"), w12_d[:])
        eye = const.tile([128, 128], BF16, tag="eye")
        nc.scalar.dma_start(eye[:], eye_d[:])
        bias_sb = const.tile([128, NQ], F32, tag="bias_sb")
        nc.scalar.dma_start(bias_sb[:], b_d[:])

        # scan-reset mask: NEG at col % 128 == 0 (runs of 128 in both phases)
        mask = const.tile([128, 2 * CPX], BF16, tag="mask")
        nc.gpsimd.memset(mask[:], 0.0)
        nc.gpsimd.memset(mask[:, 0::W], NEG)

        # resident e4m3 x (DR moving operand), [p, cc, px] row-major;
        # loaded chunkwise inside phase 1 so PE's first DR matmul
        # unblocks after one 2KB-row piece instead of the full 32KB load
        xa = persist.tile([128, NCC, HW], E4, tag="xa")

        # hw-group partial (x + x3 + x4 + bias), row-major bf16
        osb = persist.tile([128, NQ, HW], BF16, tag="osb")

        def w34_ap(d34, cc, q):
            return w34[:, (d34 * NCC + cc) * NQ + q, :]

        def w12_ap(d12, cc, q):
            return w12[:, (d12 * NCC + cc) * NQ + q, :]

        # ---- phase 1: stream (row-major; x, x3, x4 -> osb) ----
        # chunk 0 split into two 8-row halves so the first scan (and PE)
        # start after a half-size stage DMA
        p1segs = [(0, CH // 2), (CH // 2, CH // 2)]
        p1segs += [(j * CH, CH) for j in range(1, NCHUNK)]
        for h0, nrows in p1segs:
            npx = nrows * W
            st = stage_p.tile([128, NCC, CPX], BF16, tag="stage", name="st")
            for cc in range(NCC):
                nc.sync.dma_start(st[:, cc, :npx],
                                  xst_d[cc, :, h0 * W:h0 * W + npx])
            for cc in range(NCC):
                nc.sync.dma_start(xa[:, cc, h0 * W:h0 * W + npx],
                                  xa_d[:, cc, h0 * W:h0 * W + npx])
            t3 = scan_p.tile([128, NCC, CPX], BF16, tag="scan", name="t3")
            t4 = scan_p.tile([128, NCC, CPX], BF16, tag="scan", name="t4")
            if npx == CPX:
                t3f = t3[:].rearrange("p c n -> p (c n)")
                t4f = t4[:].rearrange("p c n -> p (c n)")
                stf = st[:].rearrange("p c n -> p (c n)")
                nc.vector.tensor_tensor_scan(
                    t3f, mask[:, :NCC * npx], stf, NEG, AL.add, AL.max)
                nc.vector.tensor_tensor_scan(
                    t4f[:, ::-1], mask[:, :NCC * npx], stf[:, ::-1],
                    NEG, AL.add, AL.max)
            else:
                for cc in range(NCC):
                    nc.vector.tensor_tensor_scan(
                        t3[:, cc, :npx], mask[:, :npx], st[:, cc, :npx],
                        NEG, AL.add, AL.max)
                    nc.vector.tensor_tensor_scan(
                        t4[:, cc, npx - 1::-1], mask[:, :npx],
                        st[:, cc, npx - 1::-1], NEG, AL.add, AL.max)

            for q in range(NQ):
                pt = psum_p.tile([128, CPX], F32, tag="ps")
                for r in range(nrows // SUB):
                    pr = pt[:, r * 512:(r + 1) * 512]
                    off = h0 * W + r * 512
                    for pl in range(nxpl):
                        nc.tensor.matmul(
                            pr, wx[:, pl * NQ + q, :, :], xa[:, :, off:off + 512],
                            start=(pl == 0), stop=False,
                            perf_mode=mybir.MatmulPerfMode.DoubleRow)
                    for cc in range(NCC):
                        nc.tensor.matmul(
                            pr, w34_ap(0, cc, q), t3[:, cc, r * 512:(r + 1) * 512],
                            start=False, stop=False)
                        nc.tensor.matmul(
                            pr, w34_ap(1, cc, q), t4[:, cc, r * 512:(r + 1) * 512],
                            start=False, stop=(cc == NCC - 1))
                nc.scalar.activation(
                    osb[:, q, h0 * W:h0 * W + npx], pt[:, :npx],
                    AFT.Identity, bias=bias_sb[:, q:q + 1], scale=1.0 / WS)

        # ---- phase 2: T-order (x1, x2 + merge -> out) ----
        # last band split into two 8-w halves to shrink the end-of-program
        # tail (PE/ACT/DMA trail the final scans by one segment's latency)
        segs = [(wb * CH, CH) for wb in range(NCHUNK - 1)]
        segs += [(112, 8), (120, 4), (124, 4)]
        for w0seg, ncols in segs:
            npx = ncols * H
            tt = stage_p.tile([128, NCC, CPX], BF16, tag="stage", name="tt")
            for cc in range(NCC):
                nc.scalar.dma_start(tt[:, cc, :npx],
                                    xtt_d[cc, :, w0seg * H:w0seg * H + npx])
            t1 = scan_p.tile([128, NCC, CPX], BF16, tag="scan", name="t1")
            t2 = scan_p.tile([128, NCC, CPX], BF16, tag="scan", name="t2")
            if npx == CPX:
                t1f = t1[:].rearrange("p c n -> p (c n)")
                t2f = t2[:].rearrange("p c n -> p (c n)")
                ttf = tt[:].rearrange("p c n -> p (c n)")
                nc.vector.tensor_tensor_scan(
                    t1f, mask[:, :NCC * npx], ttf, NEG, AL.add, AL.max)
                nc.vector.tensor_tensor_scan(
                    t2f[:, ::-1], mask[:, :NCC * npx], ttf[:, ::-1],
                    NEG, AL.add, AL.max)
            else:
                for cc in range(NCC):
                    nc.vector.tensor_tensor_scan(
                        t1[:, cc, :npx], mask[:, :npx], tt[:, cc, :npx],
                        NEG, AL.add, AL.max)
                    nc.vector.tensor_tensor_scan(
                        t2[:, cc, npx - 1::-1], mask[:, :npx],
                        tt[:, cc, npx - 1::-1], NEG, AL.add, AL.max)

            ot = outs_p.tile([128, NQ, CPX], BF16, tag="outs", name="ot")
            for q in range(NQ):
                pt = psum_p.tile([128, CPX], F32, tag="ps")
                for r in range(ncols // SUB):
                    pr = pt[:, r * 512:(r + 1) * 512]
                    w0 = w0seg + r * SUB
                    for cc in range(NCC):
                        nc.tensor.matmul(
                            pr, w12_ap(0, cc, q), t1[:, cc, r * 512:(r + 1) * 512],
                            start=(cc == 0), stop=False)
                        nc.tensor.matmul(
                            pr, w12_ap(1, cc, q), t2[:, cc, r * 512:(r + 1) * 512],
                            start=False, stop=False)
                    # merge: 256*I @ osb (strided rm -> T view), closes group
                    nc.tensor.matmul(
                        pr.rearrange("p (w h) -> p w h", h=H),
                        eye[:],
                        osb[:, q, :].rearrange("p (h w) -> p w h", w=W)[:, w0:w0 + SUB, :],
                        start=False, stop=True)
                nc.scalar.activation(ot[:, q, :npx], pt[:, :npx], AFT.Copy,
                                     scale=1.0 / WS)
                nc.sync.dma_start(
                    out_d[q, :, w0seg * H:w0seg * H + npx], ot[:, q, :npx])

    nc.finalize()
    return nc


_PROGRAM = None


def _get_program():
    global _PROGRAM
    if _PROGRAM is None:
        _PROGRAM = build_program()
    return _PROGRAM


def make_in_maps(x, conv_w, conv_b):
    w = np.asarray(conv_w, dtype=np.float32)
    Wx, W1, W2, W3, W4 = (w[:, i * C:(i + 1) * C] for i in range(5))

    def planes(Wm):
        # [2*NCC*NQ, 128, 128]: index (d, cc, q) -> lhsT[p, o] = WS*Wm[q*128+o, cc*128+p]
        return None  # built below per direction pair

    def pack_pair(Wa, Wb):
        out = np.empty((128, 2 * NCC * NQ, 128), dtype=np.float32)
        for d, Wm in enumerate((Wa, Wb)):
            WT = (WS * Wm).T  # [c, o]
            for cc in range(NCC):
                for q in range(NQ):
                    out[:, (d * NCC + cc) * NQ + q, :] = WT[cc * 128:(cc + 1) * 128,
                                                            q * 128:(q + 1) * 128]
        return out.reshape(128, -1).astype(ml_dtypes.bfloat16)

    w34 = pack_pair(W3, W4)
    w12 = pack_pair(W1, W2)

    # x-term DR stationaries, e4m3, scaled: [nxpl*NQ, 128, NCC*128]
    WxT = (WS * Wx).T  # [c, o]
    hi = WxT.astype(ml_dtypes.float8_e4m3)
    lo = (WxT - hi.astype(np.float32)).astype(ml_dtypes.float8_e4m3)
    wx = np.empty((128, NXPL * NQ, NCC, 128), dtype=ml_dtypes.float8_e4m3)
    for pl, Wp in enumerate((hi, lo)[:NXPL]):
        for q in range(NQ):
            for cc in range(NCC):
                wx[:, pl * NQ + q, cc, :] = Wp[cc * 128:(cc + 1) * 128,
                                               q * 128:(q + 1) * 128]
    wx = wx.reshape(128, -1)

    eye = (WS * np.eye(128)).astype(ml_dtypes.bfloat16)
    bias = np.asarray(conv_b, dtype=np.float32).reshape(NQ, 128).T.copy()  # [p, q]

    in_maps = []
    for i in range(B):
        xi = np.asarray(x[i], dtype=np.float32)  # [C, H, W]
        xst = xi.reshape(NCC, 128, HW).astype(ml_dtypes.bfloat16)
        xtt = np.ascontiguousarray(xi.transpose(0, 2, 1)).reshape(
            NCC, 128, HW).astype(ml_dtypes.bfloat16)
        xa = np.ascontiguousarray(
            xi.reshape(NCC, 128, HW).transpose(1, 0, 2)).astype(ml_dtypes.float8_e4m3)
        in_maps.append({
            "xst": xst, "xtt": xtt, "xa": xa,
            "w34": w34, "w12": w12, "wx": wx,
            "eye": eye, "bias": bias,
        })
    return in_maps


def kernel(x, conv_w, conv_b):
    nc = _get_program()
    in_maps = make_in_maps(x, conv_w, conv_b)
    res = run_bass_kernel_spmd(nc, in_maps, core_ids=list(range(B)))
    outs = []
    for i in range(B):
        o = res.results[i]["out"].astype(np.float32)  # [NQ, 128, HW] T-order (w,h)
        o = o.reshape(NQ, 128, W, H).transpose(0, 1, 3, 2).reshape(O, H, W)
        outs.append(o)
    return np.stack(outs, axis=0).astype(np.float32)
